# revision 46
# baseline (speedup 1.0000x reference)
"""Trainium2 Bass kernel: memory-slot cross-attention (nn_LocalConstructorMulti).

Reference computation (per batch b):
    Q  = memory_slots @ Wq.T                      [slots, BD]    (shared over b)
    K  = hs_b @ Wk.T                              [S, BD]
    V  = hs_b @ Wv.T                              [S, BD]
    s  = (Q_h . K_h) / sqrt(HD)  + mask           [heads, slots, S]
    p  = softmax(s, axis=S)
    o  = p @ V_h                                  [heads, slots, HD]
    y  = concat_h(o) @ Wo.T                       [slots, HID]

NEITHER PROJECTION IS EVER MATERIALIZED.  Both big GEMMs fold away by
associativity around the tiny slot dimension (8 slots x 8 heads = 64):

    scores = Q.(Wk hs^T) = (Q @ Wk) . hs^T        qk: [64, HID], host, 16 MF
    o_num  = p^T (hs Wv^T) = (p^T hs) Wv^T        u:  [HID, 64] on device

That replaces 2 x 4.3 GMAC/core of K/V projections with 2 x 0.54 GMAC of
skinny contractions against hs -- the kernel becomes DMA-bound at the
~332 GB/s HBM stream (cost-model timeline: v1 413.7us -> 93.6us, with the
PE, ACT and DVE engines hiding entirely under the DMA stream).

Sharding: 8 cores = 4 batches x 2 sequence-halves.  Each core holds its
2048-row half in BOTH orientations: 8 MB fp8 hsT for the k-contracted
score pass (score-side fp8 noise attenuates ~64x through the near-uniform
softmax; the stationary is fp8 while the moving qk stays bf16) and 16 MB
bf16 row-major for the s-contracted u pass (u noise reaches the output
directly, so it keeps full bf16).  Per core, for all 8 heads:

    phase 1: s = qk . hsT_half   (stationary hsT blocks, moving qk [128,64];
             mask fused as per-partition Exp bias -> p, all local)
    phase 2: u = p^T hs_half     (stationary row-major hs blocks, moving p;
             den = 1^T p via a ones-column stationary)
    phase 3: z = u @ Wv^T        (32 mov-512 matmuls on the aggregated u)

The host sums the two halves' linear partials (z, den), normalizes, and
applies the 67 MFLOP o_proj (0.05% of the model's FLOPs).  The exp
nonlinearity is the only thing pinning p to the device between the passes;
everything that touches the 256 MB hs tensor stays on-device.

Layout notes: both hs orientations are host-preshuffled into
[blocks, 128 part, subtiles, 512] so every DMA has multi-KiB contiguous
per-partition lines and chains start on pair-group chunk arrival; every
PSUM accumulator owns a full bank or shares one only with other PE-W
accumulation groups (PE-W + DVE/ACT-R same-bank erratum); the score pass
streams first so exp'd p tiles are ready exactly when the u pass needs
them; wv/qk/mb ride outside the hs stream.
"""

import sys

if "/opt/trn_rl_repo" not in sys.path:
    sys.path.insert(0, "/opt/trn_rl_repo")

import ml_dtypes
import numpy as np

import concourse.bass as bass  # noqa: F401  (AP helpers)
import concourse.mybir as mybir
import concourse.tile as tile
from concourse import bacc
from concourse.bass_utils import run_bass_kernel_spmd

BF16 = mybir.dt.bfloat16
FP8 = mybir.dt.float8e4
F32 = mybir.dt.float32
npbf16 = ml_dtypes.bfloat16
npfp8 = ml_dtypes.float8_e4m3

B, S, HID = 4, 4096, 4096
SLOTS, HEADS, BD = 8, 8, 512
HD = BD // HEADS  # 64
N_CORES = 8
HALVES = N_CORES // B  # sequence halves per batch
SL = S // HALVES  # 2048 local rows
HSL = HEADS * SLOTS  # 64 head-slot columns
MASK_NEG = -30000.0

NKS = HID // 128  # 32 k-subtiles
NRT = SL // 128  # 16 local row tiles
TBLK = 4  # hsT column blocks (512 rows each)
RBLK = 8  # row-major k blocks (512 k-cols each)

# test.py can flip this to capture an NTFF profile; harness never touches it.
TRACE = False
TRACE_CORES = None
LAST_RESULT = None

_cache = {}


def _build_module():
    """Emit + compile the single-core Bass module (same NEFF on all cores)."""
    nc = bacc.Bacc("TRN2", target_bir_lowering=False, debug=False, num_devices=N_CORES)

    hstT = nc.dram_tensor("hstT", [TBLK, 128, NKS, SL // TBLK], FP8, kind="ExternalInput").ap()
    hsrT = nc.dram_tensor("hsrT", [RBLK, 128, NRT, HID // RBLK], BF16, kind="ExternalInput").ap()
    qkT = nc.dram_tensor("qkT", [128, NKS, HSL], BF16, kind="ExternalInput").ap()
    wvT = nc.dram_tensor("wvT", [128, NKS, BD], BF16, kind="ExternalInput").ap()
    mbT = nc.dram_tensor("mbT", [128, NRT], F32, kind="ExternalInput").ap()
    zT = nc.dram_tensor("zT", [HSL, BD], F32, kind="ExternalOutput").ap()
    denT = nc.dram_tensor("denT", [1, HSL], F32, kind="ExternalOutput").ap()

    CT = SL // TBLK  # 512 score columns per hsT block
    CR = HID // RBLK  # 512 k columns per row-major block

    with tile.TileContext(nc) as tc:
        with (
            tc.tile_pool(name="consts", bufs=1) as consts,
            tc.tile_pool(name="hstp", bufs=2) as hstp,
            tc.tile_pool(name="hsrp", bufs=2) as hsrp,
        ):
            # ---- resident operands ---------------------------------------
            qk_sb = consts.tile([128, NKS, HSL], BF16)
            wv_sb = consts.tile([128, NKS, BD], BF16)
            mb_sb = consts.tile([128, NRT], F32)
            ones_sb = consts.tile([128, 1], BF16)
            nc.vector.memset(ones_sb, 1.0)

            # ---- persistent intermediates --------------------------------
            pt_sb = consts.tile([128, NRT, HSL], BF16)  # exp(scores)
            u_sb = consts.tile([128, NKS, HSL], BF16)  # u = p^T hs
            den_sb = consts.tile([1, HSL], F32)
            z_sb = consts.tile([HSL, BD], F32)

            with (
                tc.tile_pool(name="sps", bufs=2, space="PSUM") as sps,
                tc.tile_pool(name="ups", bufs=2, space="PSUM") as ups,
                tc.tile_pool(name="dps", bufs=1, space="PSUM") as dps,
                tc.tile_pool(name="zps", bufs=1, space="PSUM") as zps,
            ):
                # ---- phase 1: scores + exp from the k-partitioned half ---
                for blk in range(TBLK):
                    hst_t = hstp.tile([128, NKS, CT], FP8, tag="hst")
                    for q in range(8):
                        h = NKS // 8
                        if blk == 0 and q < 4:
                            # qk/mb ride with the first chunks; they are
                            # tiny and needed by the first chains
                            if q == 0:
                                nc.sync.dma_start(out=qk_sb, in_=qkT)
                                nc.sync.dma_start(out=mb_sb, in_=mbT)
                        nc.sync.dma_start(
                            out=hst_t[:, q * h : (q + 1) * h],
                            in_=hstT[blk][:, q * h : (q + 1) * h],
                        )
                    if blk == TBLK - 1:
                        # wv is first needed by phase 3
                        nc.sync.dma_start(out=wv_sb, in_=wvT)
                    for r in range(CT // 128):
                        rt = blk * (CT // 128) + r
                        s_ps = sps.tile([128, 512], F32, tag="s")
                        for k in range(NKS):
                            nc.tensor.matmul(
                                s_ps[:, 0:HSL],
                                hst_t[:, k, r * 128 : (r + 1) * 128],
                                qk_sb[:, k, :],
                                start=(k == 0),
                                stop=(k == NKS - 1),
                            )
                        nc.scalar.activation(
                            out=pt_sb[:, rt, :],
                            in_=s_ps[:, 0:HSL],
                            func=mybir.ActivationFunctionType.Exp,
                            bias=mb_sb[:, rt : rt + 1],
                            scale=1.0,
                        )

                # den = 1^T p  (one ones-column stationary chain)
                den_ps = dps.tile([128, 512], F32)
                for rt in range(NRT):
                    nc.tensor.matmul(
                        den_ps[0:1, 0:HSL],
                        ones_sb,
                        pt_sb[:, rt, :],
                        start=(rt == 0),
                        stop=(rt == NRT - 1),
                    )
                nc.scalar.copy(out=den_sb, in_=den_ps[0:1, 0:HSL])
                nc.sync.dma_start(out=denT, in_=den_sb)

                # ---- phase 2: u = p^T hs from the row-major half ---------
                # stationary row-major hs blocks [128 s, 128 k], moving p
                for blk in range(RBLK):
                    hsr_t = hsrp.tile([128, NRT, CR], BF16, tag="hsr")
                    for q in range(4):
                        h = NRT // 4
                        nc.sync.dma_start(
                            out=hsr_t[:, q * h : (q + 1) * h],
                            in_=hsrT[blk][:, q * h : (q + 1) * h],
                        )
                    u_ps = ups.tile([128, 512], F32, tag="u")
                    for kc in range(CR // 128):
                        for ss in range(NRT):
                            nc.tensor.matmul(
                                u_ps[:, kc * 128 : kc * 128 + HSL],
                                hsr_t[:, ss, kc * 128 : (kc + 1) * 128],
                                pt_sb[:, ss, :],
                                start=(ss == 0),
                                stop=(ss == NRT - 1),
                            )
                    for kc in range(CR // 128):
                        ks = blk * (CR // 128) + kc
                        eng = nc.vector.tensor_copy if kc % 2 else nc.scalar.copy
                        eng(
                            out=u_sb[:, ks, :],
                            in_=u_ps[:, kc * 128 : kc * 128 + HSL],
                        )

                # ---- phase 3: z = u @ Wv^T (aggregated, tiny) ------------
                z_ps = zps.tile([HSL, BD], F32)
                for k in range(NKS):
                    nc.tensor.matmul(
                        z_ps,
                        u_sb[:, k, :],
                        wv_sb[:, k, :],
                        start=(k == 0),
                        stop=(k == NKS - 1),
                    )
                nc.scalar.copy(out=z_sb, in_=z_ps)
                nc.sync.dma_start(out=zT, in_=z_sb)

    nc.compile()
    return nc


def _get_module():
    if "m" not in _cache:
        _cache["m"] = _build_module()
    return _cache["m"]


def _prep_in_maps(hs, mask, ms, Wq, Wk, Wv, Wo):
    """Shard the full inputs into 8 per-core input maps (host-side)."""
    Q = (ms @ Wq.T).astype(np.float32)  # [SLOTS, BD]
    # qk[h*8+n, :] = (Q_h[n, :] @ Wk_h) / sqrt(HD)
    qk = np.empty((HSL, HID), np.float32)
    for h in range(HEADS):
        qk[h * SLOTS : (h + 1) * SLOTS] = (
            Q[:, h * HD : (h + 1) * HD] @ Wk[h * HD : (h + 1) * HD, :]
        ) * np.float32(1.0 / np.sqrt(HD))
    # [HID, HSL] -> [128 ki, NKS, HSL]
    qkc = np.ascontiguousarray(
        qk.T.reshape(NKS, 128, HSL).transpose(1, 0, 2).astype(npbf16)
    )
    wvc = np.ascontiguousarray(
        Wv.T.reshape(NKS, 128, BD).transpose(1, 0, 2).astype(npbf16)
    )

    in_maps = []
    for c in range(N_CORES):
        b, half = c // HALVES, c % HALVES
        rows = slice(half * SL, (half + 1) * SL)
        hsh = hs[b][rows]  # [SL, HID] f32
        # hsT half: [HID, SL] -> [TBLK, 128 ki, NKS, CT]
        hst = (
            hsh.T.reshape(NKS, 128, TBLK, SL // TBLK)
            .transpose(2, 1, 0, 3)
        )
        # row-major half: [SL, HID] -> [RBLK, 128 si, NRT, CR]
        hsr = (
            hsh.reshape(NRT, 128, RBLK, HID // RBLK)
            .transpose(2, 1, 0, 3)
        )
        mb = (
            np.where(mask[b][rows] == 0, np.float32(MASK_NEG), np.float32(0.0))
            .astype(np.float32)
            .reshape(NRT, 128)
            .T
        )
        in_maps.append(
            {
                "hstT": np.ascontiguousarray(hst.astype(npfp8)),
                "hsrT": np.ascontiguousarray(hsr.astype(npbf16)),
                "qkT": qkc,
                "wvT": wvc,
                "mbT": np.ascontiguousarray(mb),
            }
        )
    return in_maps


def kernel(hidden_states, attention_mask, memory_slots, Wq, Wk, Wv, Wo):
    global LAST_RESULT
    hs = np.asarray(hidden_states, dtype=np.float32)
    mask = np.asarray(attention_mask)
    ms = np.asarray(memory_slots, dtype=np.float32)
    Wq = np.asarray(Wq, dtype=np.float32)
    Wk = np.asarray(Wk, dtype=np.float32)
    Wv = np.asarray(Wv, dtype=np.float32)
    Wo = np.asarray(Wo, dtype=np.float32)

    nc = _get_module()
    in_maps = _prep_in_maps(hs, mask, ms, Wq, Wk, Wv, Wo)

    kwargs = {}
    if TRACE:
        kwargs = {"trace": True}
        if TRACE_CORES is not None:
            kwargs["trace_cores"] = TRACE_CORES
    res = run_bass_kernel_spmd(nc, in_maps, core_ids=list(range(N_CORES)), **kwargs)
    LAST_RESULT = res

    # host combine: sum the two halves' linear partials, normalize per
    # (head, slot), apply the tiny o_proj (67 MFLOP)
    WoH = Wo.reshape(HID, HEADS, HD)  # [out, h, d]
    y = np.empty((B, SLOTS, HID), np.float32)
    for b in range(B):
        z = res.results[2 * b]["zT"] + res.results[2 * b + 1]["zT"]  # [64, 512]
        den = (
            res.results[2 * b]["denT"] + res.results[2 * b + 1]["denT"]
        ).reshape(HEADS, SLOTS)
        o = z.reshape(HEADS, SLOTS, BD)  # [h, n, bd]
        oh = np.empty((SLOTS, HEADS, HD), np.float32)
        for h in range(HEADS):
            oh[:, h, :] = o[h, :, h * HD : (h + 1) * HD] / den[h][:, None]
        y[b] = np.einsum("nhd,ohd->no", oh, WoH)
    return np.ascontiguousarray(y.astype(np.float32))
